# revision 1
# baseline (speedup 1.0000x reference)
"""AttentionFusion kernel for 8 TRN2 NeuronCores.

Reference computation:
    expanded_video = repeat_interleave(video, 20, dim=1)        # [B, 1280, D]
    scores = expanded_video @ text.T * D**-0.5                  # [B, 1280, 256]
    attn_out = softmax(scores) @ text                           # [B, 1280, D]
    out = concat([text, video, expanded_video + attn_out], 1)   # [B, 1600, D]

Key algebraic fact: repeated (identical) query rows produce identical
attention outputs, so only the 64 unique video rows per batch need
attention; the 20x replication happens on the host during unsharding.

Sharding (v5): one core PAIR per batch, ZERO cross-core traffic.
Both cores of a pair redundantly compute stage 1 + softmax over the
FULL 256 text rows (cheap: fp8 operands, 2.6 MB), and each core
computes stage 2 for ITS d-half only - which halves the expensive
bf16 T-natural operand (2.6 MB instead of 5.2). Per-core HBM bytes
drop to ~6.6 MB; the per-pair HBM stack (716 GB/s shared by 2 cores)
is the binding resource. A 33 KB pair AllGather alternative measured
~30 us of fixed collective latency on this runtime, so redundant
compute wins.

Host pre-transposes inputs into the layouts the TensorEngine needs
(contraction dim on partitions), so every DMA is contiguous.
"""

import sys

import numpy as np

if "/opt/trn_rl_repo" not in sys.path:
    sys.path.insert(0, "/opt/trn_rl_repo")

import ml_dtypes

REPEAT = 20
D = 10240
DH = D // 2       # d-half: stage-2 output columns per core
SCALE = D ** (-0.5)
B, TT, TV = 4, 256, 64
NCORES = 8
KH = 128          # k-half: text rows per core in stage 1
DJ = 80           # number of 128-wide d chunks (stage-1 contraction tiles)
KT = 2            # number of 128-wide k tiles (stage-2 contraction)
NR = 5            # stage-2 rounds; each = 2 col groups x 512 cols x 2 kt
TT_CHUNK = 20     # stage-1 j's per input DMA chunk

_compiled = None


def _build():
    import concourse.mybir as mybir
    import concourse.tile as tile
    from concourse import bacc
    from concourse.masks import make_identity

    f32 = mybir.dt.float32
    bf16 = mybir.dt.bfloat16
    fp8 = mybir.dt.float8e3

    nc = bacc.Bacc(
        "TRN2", target_bir_lowering=False, debug=False, num_devices=NCORES
    )
    qtt_h = nc.dram_tensor("qtt", [128, DJ, TV + TT], fp8, kind="ExternalInput")
    tn_h = nc.dram_tensor("tn", [128, KT, DH], bf16, kind="ExternalInput")
    out_h = nc.dram_tensor("out", [128, NR * 512], bf16, kind="ExternalOutput")

    with tile.TileContext(nc) as tc:
        with (
            tc.tile_pool(name="ttp", bufs=4) as tt_pool,
            tc.tile_pool(name="tnp", bufs=2) as tn_pool,
            tc.tile_pool(name="smp", bufs=1) as sm_pool,
            tc.tile_pool(name="osp", bufs=NR) as os_pool,
            tc.tile_pool(name="ps_p", bufs=1, space="PSUM") as ps_p_pool,
            tc.tile_pool(name="ps_s", bufs=1, space="PSUM") as ps_s_pool,
            tc.tile_pool(name="ps_w", bufs=2, space="PSUM") as ps_w_pool,
            tc.tile_pool(name="ps_o", bufs=2, space="PSUM") as ps_o_pool,
        ):
            ident = sm_pool.tile([TV, TV], bf16, tag="ident")
            make_identity(nc, ident[:])
            # sel[64g+q, q] = 1: partition-halves reducer for stage-1 partials
            sel = sm_pool.tile([128, TV], bf16, tag="sel")
            nc.gpsimd.memset(sel[:], 0.0)
            for g in range(2):
                make_identity(nc, sel[g * TV : (g + 1) * TV, :], nomemset=True)

            # stage 1: S = Q @ T.T, 2x col-tiled by j parity; qt and tt
            # stream together in one packed chunk sequence (early PE start)
            ps_p = ps_p_pool.tile([128, TT], f32)
            for c in range(DJ // TT_CHUNK):
                qtt_sb = tt_pool.tile([128, TT_CHUNK, TV + TT], fp8)
                nc.sync.dma_start(
                    qtt_sb[:], qtt_h[:, c * TT_CHUNK : (c + 1) * TT_CHUNK, :]
                )
                for j in range(TT_CHUNK):
                    jj = c * TT_CHUNK + j
                    ge = jj % 2
                    nc.tensor.matmul(
                        ps_p[ge * TV : (ge + 1) * TV, :],
                        lhsT=qtt_sb[:, j, 0:TV],
                        rhs=qtt_sb[:, j, TV : TV + TT],
                        start=(jj < 2),
                        stop=(jj >= DJ - 2),
                        tile_position=(0, ge * TV),
                        skip_group_check=True,
                    )

            # stage-2 operand streams in while stage 1 runs
            tn_sb = []
            for r in range(2):
                t = tn_pool.tile([128, KT, DH // 2], bf16)
                nc.sync.dma_start(
                    t[:], tn_h[:, :, r * (DH // 2) : (r + 1) * (DH // 2)]
                )
                tn_sb.append(t)

            # reduce the two j-parity partials: S = sel.T @ partials
            sp_sb = sm_pool.tile([128, TT], bf16, tag="sp")
            nc.scalar.copy(sp_sb[:], ps_p[:])
            ps_s = ps_s_pool.tile([TV, TT], f32)
            nc.tensor.matmul(ps_s[:], lhsT=sel[:], rhs=sp_sb[:])

            # softmax along k (the free dim of ps_s)
            mx = sm_pool.tile([TV, 1], f32, tag="mx")
            nc.vector.reduce_max(mx[:], ps_s[:], axis=mybir.AxisListType.X)
            negb = sm_pool.tile([TV, 1], f32, tag="negb")
            nc.scalar.mul(negb[:], mx[:], -SCALE)
            e = sm_pool.tile([TV, TT], f32, tag="e")
            lsum = sm_pool.tile([TV, 1], f32, tag="lsum")
            nc.scalar.activation(
                e[:],
                ps_s[:],
                mybir.ActivationFunctionType.Exp,
                bias=negb[:],
                scale=SCALE,
                accum_out=lsum[:],
            )
            rl = sm_pool.tile([TV, 1], f32, tag="rl")
            nc.vector.reciprocal(rl[:], lsum[:])
            w = sm_pool.tile([TV, TT], bf16, tag="w")
            nc.vector.tensor_scalar_mul(w[:], e[:], rl[:])

            # W[64, 256] -> WT[128, 2, 64] (k on partitions) via PE transpose
            wt_sb = sm_pool.tile([128, KT, TV], bf16, tag="wt")
            for kt in range(KT):
                wt_ps = ps_w_pool.tile([128, TV], bf16)
                nc.tensor.transpose(
                    wt_ps[:], w[:, kt * 128 : (kt + 1) * 128], ident[:]
                )
                nc.scalar.copy(wt_sb[:, kt, :], wt_ps[:])

            # stage 2: O[:, d-half] = W @ T[:, d-half], 2x column-tiled
            for r in range(NR):
                ps_o = ps_o_pool.tile([128, 512], f32)
                for g2 in range(2):
                    n = 2 * r + g2
                    src_t = tn_sb[n // 5]
                    off = (n % 5) * 512
                    for kt in range(KT):
                        nc.tensor.matmul(
                            ps_o[g2 * TV : (g2 + 1) * TV, :],
                            lhsT=wt_sb[:, kt, :],
                            rhs=src_t[:, kt, off : off + 512],
                            start=(kt == 0),
                            stop=(kt == KT - 1),
                            tile_position=(0, g2 * TV),
                        )
                osb = os_pool.tile([128, 512], bf16)
                nc.scalar.copy(osb[:], ps_o[:])
                nc.sync.dma_start(out_h[:, r * 512 : (r + 1) * 512], osb[:])

    nc.compile()
    return nc


def _prepare_in_maps(text, video):
    tb = np.asarray(text, dtype=np.float32).astype(ml_dtypes.bfloat16)
    t8 = np.asarray(text, dtype=np.float32).astype(ml_dtypes.float8_e3m4)
    v8 = np.asarray(video, dtype=np.float32).astype(ml_dtypes.float8_e3m4)
    in_maps = []
    for c in range(NCORES):
        b, h = divmod(c, 2)
        # qtt[p, j, 0:64] = video[b, q, j*128+p]; [p, j, 64+k] = text[b, k, j*128+p]
        qtt = np.empty((128, DJ, TV + TT), dtype=ml_dtypes.float8_e3m4)
        qtt[:, :, :TV] = v8[b].reshape(TV, DJ, 128).transpose(2, 1, 0)
        qtt[:, :, TV:] = t8[b].reshape(TT, DJ, 128).transpose(2, 1, 0)
        # tn[p, kt, d'] = text[b, kt*128+p, h*5120+d']
        tn = np.ascontiguousarray(
            tb[b, :, h * DH : (h + 1) * DH].reshape(KT, 128, DH).transpose(1, 0, 2)
        )
        in_maps.append({"qtt": qtt, "tn": tn})
    return in_maps


def _assemble(results, text, video):
    tf = np.asarray(text, dtype=np.float32)
    vf = np.asarray(video, dtype=np.float32)
    attn = np.empty((B, TV, D), np.float32)
    for c in range(NCORES):
        b, h = divmod(c, 2)
        o128 = np.asarray(results[c]["out"], dtype=np.float32)
        # out128[64*g2+q, r*512+x] = O[q, h*5120 + (2r+g2)*512 + x]
        o = o128.reshape(2, TV, NR, 512).transpose(1, 2, 0, 3).reshape(TV, DH)
        attn[b, :, h * DH : (h + 1) * DH] = o
    fused = vf + attn
    return np.concatenate([tf, vf, np.repeat(fused, REPEAT, axis=1)], axis=1)


def _ensure_ntff_hook():
    """Register the axon NTFF profiling hook if the image lacks
    antenv.axon_hooks (trace=True degrades to no-op otherwise)."""
    import types

    try:
        from antenv import axon_hooks  # noqa: F401

        return
    except ImportError:
        pass
    mod = types.ModuleType("antenv.axon_hooks")
    _hook = [None]
    mod.set_axon_ntff_profile_hook = lambda h: _hook.__setitem__(0, h)
    mod.get_axon_ntff_profile_hook = lambda: _hook[0]
    sys.modules["antenv.axon_hooks"] = mod
    import antenv

    antenv.axon_hooks = mod
    try:
        from trn_agent_boot.trn_boot import _ntff_profile_via_ctypes

        mod.set_axon_ntff_profile_hook(
            _ntff_profile_via_ctypes("/opt/axon/libaxon_pjrt.so")
        )
    except Exception:
        pass


def _run(text_features, video_features, trace=False, **spmd_kwargs):
    global _compiled
    if _compiled is None:
        _compiled = _build()
    if trace:
        _ensure_ntff_hook()
    from concourse.bass_utils import run_bass_kernel_spmd

    in_maps = _prepare_in_maps(text_features, video_features)
    res = run_bass_kernel_spmd(
        _compiled,
        in_maps,
        core_ids=list(range(NCORES)),
        trace=trace,
        **spmd_kwargs,
    )
    out = _assemble(res.results, text_features, video_features)
    return out, res


def kernel(text_features, video_features):
    out, _ = _run(text_features, video_features)
    return out

